# revision 39
# baseline (speedup 1.0000x reference)
"""Trainium2 Bass kernel for nn_CIFARClassifier (8-block dense CNN, C=3).

Sharding: pure data parallel — batch 4096 split as 512 images per core
across 8 NeuronCores; the tiny weights/BN params are replicated (folded
host-side into per-block conv matrices + bias vectors).

Per-core layout: activations live in SBUF as [(c,h) partitions, (b,w) free]
with w padded by one zero column on each side (SAME-conv padding), all in
bf16 (PE runs bf16 at 1 cycle/row vs fp32's 4; PSUM accumulation and the
softmax tail stay f32 — rel err ~2e-3 vs the 2e-2 gate).  The h index is
placed with its low bits as the HIGH partition bits
(r32(c,h) = (h&1)*64 + ((h>>1)&1)*32 + c*8 + (h>>2)), so each 2x2 maxpool is
a free-dim max over w-pairs plus a partition-half max over h-pairs; the
h-half realignment is one contiguous SBUF->SBUF DMA (walrus requires both
SBUF operands of a DVE op to share a base partition, so the high half is
DMA'd down to partition 0 first).

A 3x3 conv = per-kx PE matmuls (PSUM-accumulated, rhs shifted by kx into
the padded columns); the stationary operand is a host-built KxM matrix
encoding (ic,ky)->(oc,ho) mixing for all h rows at once, BN folded in
(scale) with an ACT-fused relu(x+bias) (shift).  Matmul time is rows-only
(K never matters), so where K is small the kx taps are K-STACKED: the
16x16 stage packs kx=0,1 into one K=112 matmul, the 8x8 stage packs all
three into K=72 — the moving operand carries shifted replicas of the
input on higher partition groups, built by flat one-descriptor-per-
partition SBUF->SBUF DMAs (idle DMA engines; the b-boundary wrap lands in
columns the matmul never reads).  This cuts PE rows by ~28% vs 3 passes
everywhere.  GAP(1/64) + the final 1x1 conv fold into one [24,10] matmul
whose lhsT is the data — logits land as [batch, 10] for the log-softmax
tail.

Activation tensors are PERSISTENT tiles (explicit two-buffer sets keyed by
sub-tile parity, not pool rotations): conv/pool writers touch only
interior columns and residual adds rewrite the full padded width, so pad
columns are zeroed exactly once at startup (the race detector also accepts
pad reads only for same-tensor history).

Sync discipline: this container's walrus accepts at most ONE sem-wait per
instruction, so the BIR is post-processed before compile — extra waits are
split into single-wait EventSemaphore instructions on the same engine
(_split_multiwait).  Performance shape: constants are packed into 4 DMAs,
x is host-pre-permuted (bf16) so each sub-tile load is one contiguous DMA,
the four batch sub-tiles are emitted pairwise-interleaved (per-PSUM-chunk
yields) with the twin offset by 5 rounds — engine queues are in-order, so
a stall can only be covered by work emitted at that queue position, and
in lockstep both twins stall on their pool chains simultaneously — and
residual adds/GAP run per-chunk right behind each conv's PSUM drain.
Measured on trn2 (NTFF profile): ~179us/core (run-to-run clock/throttle
regime adds up to ~18% variance), vs 624us for the fp32 3-matmul
PE-realign version this evolved from.  Residual adds write interior
columns only; pad columns of every padded buffer are startup-zeroed via
one strided memset each and never rewritten.  Maxpool chains run in two
64-image halves so DVE/DMA/DVE steps pipeline; pair B's x loads are
prefetched mid-pair-A on the idle Sync queue.  Tried and reverted as
non-improvements at matched clock regimes: 4-way/staggered sub-tile
interleave (in-order queues + power throttle), 2-bank PSUM chunks
(dependency convoys), chunk-paired residual adds, batched softmax tail.
"""

import numpy as np

EPS = 1e-5
B_TOTAL = 4096
N_CORES = 8
B_CORE = B_TOTAL // N_CORES  # 512
NB = 128                     # batch sub-tile per inner iteration
N_SUB = B_CORE // NB         # 4
P32, P16, P8 = 120, 56, 24   # used partitions (with pool-alignment holes)


def _rmap32(c, h):
    return (h & 1) * 64 + ((h >> 1) & 1) * 32 + c * 8 + (h >> 2)


def _rmap16(c, h):
    return (h & 1) * 32 + c * 8 + (h >> 1)


def _rmap8(c, h):
    return c * 8 + h


def _conv_mats(wp, rmap, R, P):
    """wp: [oc=3, ic=3, ky=3, kx=3] BN-folded weights -> [kx, K=P, M=P]."""
    mats = np.zeros((3, P, P), np.float32)
    for oc in range(3):
        for ho in range(R):
            m = rmap(oc, ho)
            for ic in range(3):
                for ky in range(3):
                    hi = ho + ky - 1
                    if 0 <= hi < R:
                        k = rmap(ic, hi)
                        mats[:, k, m] = wp[oc, ic, ky, :]
    return mats


def _build_consts(ws, w9, gammas, betas, means, variances):
    ws = np.asarray(ws, np.float64)
    w9 = np.asarray(w9, np.float64)
    cm32 = np.zeros((2, 3, P32, P32), np.float32)
    cm16 = np.zeros((3, 3, P16, P16), np.float32)
    cm8 = np.zeros((3, 3, P8, P8), np.float32)
    bias32 = np.zeros((2, P32), np.float32)
    bias16 = np.zeros((3, P16), np.float32)
    bias8 = np.zeros((3, P8), np.float32)
    for blk in range(8):
        inv = np.asarray(gammas[blk], np.float64) / np.sqrt(
            np.asarray(variances[blk], np.float64) + EPS
        )
        wp = ws[blk] * inv[:, None, None, None]
        bb = np.asarray(betas[blk], np.float64) - np.asarray(means[blk], np.float64) * inv
        if blk < 2:
            cm32[blk] = _conv_mats(wp, _rmap32, 32, P32)
            for oc in range(3):
                for h in range(32):
                    bias32[blk, _rmap32(oc, h)] = bb[oc]
        elif blk < 5:
            cm16[blk - 2] = _conv_mats(wp, _rmap16, 16, P16)
            for oc in range(3):
                for h in range(16):
                    bias16[blk - 2, _rmap16(oc, h)] = bb[oc]
        else:
            cm8[blk - 5] = _conv_mats(wp, _rmap8, 8, P8)
            for oc in range(3):
                for h in range(8):
                    bias8[blk - 5, _rmap8(oc, h)] = bb[oc]
    import ml_dtypes
    bf16 = ml_dtypes.bfloat16
    ghead = np.zeros((P8, 10), np.float32)
    for c in range(3):
        for h in range(8):
            ghead[_rmap8(c, h), :] = w9[:, c, 1, 1] / 64.0
    # Pack all constants into 4 tensors (one DMA each — SWDGE issue cost is
    # ~2us per dma_start).  Conv matrices are bf16 (PE runs bf16 at 1
    # cyc/row vs fp32's 4); bias/head stay f32 (the tail is f32).
    # 16-stage: kx=0,1 are K-stacked into one [112,56] stationary (the
    # moving operand carries a shifted replica on partitions 56:112), kx=2
    # stays single.  8-stage: all three kx stack into [72,24].
    cmall32 = np.zeros((P32, 6 * P32), bf16)
    for blk in range(2):
        for kx in range(3):
            i = blk * 3 + kx
            cmall32[:, i * P32:(i + 1) * P32] = cm32[blk, kx].astype(bf16)
    cmall16 = np.zeros((2 * P16, 6 * P16), bf16)
    for blk in range(3):
        cmall16[0:P16, blk * P16:(blk + 1) * P16] = cm16[blk, 0].astype(bf16)
        cmall16[P16:2 * P16, blk * P16:(blk + 1) * P16] = cm16[blk, 1].astype(bf16)
        cmall16[0:P16, (3 + blk) * P16:(4 + blk) * P16] = cm16[blk, 2].astype(bf16)
    cmall8 = np.zeros((3 * P8, 3 * P8), bf16)
    for blk in range(3):
        for kx in range(3):
            cmall8[kx * P8:(kx + 1) * P8, blk * P8:(blk + 1) * P8] = (
                cm8[blk, kx].astype(bf16))
    # bias columns 0:8; GAP head matrix (f32) in columns 8:18
    biasall = np.zeros((P32, 18), np.float32)
    for blk in range(8):
        if blk < 2:
            biasall[:P32, blk] = bias32[blk]
        elif blk < 5:
            biasall[:P16, blk] = bias16[blk - 2]
        else:
            biasall[:P8, blk] = bias8[blk - 5]
    biasall[0:P8, 8:18] = ghead
    return {
        "cmall32": cmall32, "cmall16": cmall16, "cmall8": cmall8,
        "biasall": biasall,
    }


def build_program():
    import concourse.bass as bass
    import concourse.tile as tile
    from concourse import mybir

    f32 = mybir.dt.float32
    bf16 = mybir.dt.bfloat16
    AFT = mybir.ActivationFunctionType
    ALU = mybir.AluOpType
    AX = mybir.AxisListType

    nc = bass.Bass()
    x_d = nc.dram_tensor("x", [N_SUB, P32, NB, 34], bf16, kind="ExternalInput")
    cm32_d = nc.dram_tensor("cmall32", [P32, 6 * P32], bf16, kind="ExternalInput")
    cm16_d = nc.dram_tensor("cmall16", [2 * P16, 6 * P16], bf16,
                            kind="ExternalInput")
    cm8_d = nc.dram_tensor("cmall8", [3 * P8, 3 * P8], bf16, kind="ExternalInput")
    bias_d = nc.dram_tensor("biasall", [P32, 18], f32, kind="ExternalInput")
    out_d = nc.dram_tensor("out", [B_CORE, 10], f32, kind="ExternalOutput")

    with tile.TileContext(nc) as tc:
        with (
            tc.tile_pool(name="consts", bufs=1) as cpool,
            tc.tile_pool(name="acts", bufs=1) as apool,
            tc.tile_pool(name="ps", bufs=2, space="PSUM") as pspool,
            tc.tile_pool(name="ph", bufs=2, space="PSUM") as phpool,
            tc.tile_pool(name="small", bufs=4) as spool,
            tc.tile_pool(name="resp", bufs=1) as rpool,
        ):
            # ---- constants: 4 packed tiles, 4 DMAs (issued below,
            # after sub-tile 0's first x half-load) ----
            cma32 = cpool.tile([P32, 6 * P32], bf16, tag="cma32")
            cma16 = cpool.tile([2 * P16, 6 * P16], bf16, tag="cma16")
            cma8 = cpool.tile([3 * P8, 3 * P8], bf16, tag="cma8")
            biasa = cpool.tile([P32, 18], f32, tag="biasa")

            cm32_t = {}
            for blk in range(2):
                for kx in range(3):
                    i = blk * 3 + kx
                    cm32_t[(blk, kx)] = cma32[:, i * P32:(i + 1) * P32]
            cm16f_t = {}
            cm16s_t = {}
            for b in range(3):
                cm16f_t[b] = cma16[0:2 * P16, b * P16:(b + 1) * P16]
                cm16s_t[b] = cma16[0:P16, (3 + b) * P16:(4 + b) * P16]
            cm8f_t = {}
            for b in range(3):
                cm8f_t[b] = cma8[0:3 * P8, b * P8:(b + 1) * P8]
            bias_t = {}
            for blk in range(8):
                P = P32 if blk < 2 else (P16 if blk < 5 else P8)
                bias_t[blk] = biasa[0:P, blk:blk + 1]
            gh_t = biasa[0:P8, 8:18]

            res_all = rpool.tile([128, N_SUB, 10], f32, tag="res_all")

            # ---- persistent activation tiles (explicit double buffer) ----
            # Each logical activation tensor gets two persistent buffers
            # (subtile parity).  Conv/pool writers only touch interior
            # columns; residual adds rewrite the full width with zero pads;
            # so each padded buffer's pad columns are zeroed exactly once,
            # up front.  Persistent tensors (not pool rotations) keep the
            # pad bytes owned by the same tensor, which the race detector
            # accepts.
            tile_specs = {
                "x1": 34, "a2": 34, "a12": 34, "a3": 34,
                "wp": 16, "wph": 16,
                "b4": 18, "b5": 18, "b45": 18, "b6": 18, "b56": 18, "b7": 18,
                "wp2": 8, "wph2": 8,
                "c8": 10, "c9": 10, "c89": 10, "c10": 10,
            }
            pad_tags = {"a2", "a3", "b4", "b5", "b6", "b7", "c8", "c9", "c10"}
            tiles = {}
            for tag, w in tile_specs.items():
                for s in range(2):
                    t = apool.tile([128, NB, w], bf16, tag=f"{tag}_{s}")
                    tiles[(tag, s)] = t
                    if tag in pad_tags:
                        nc.vector.memset(t[:, :, 0:1], 0.0)
                        nc.vector.memset(t[:, :, w - 1:w], 0.0)

            # sub-tile 0's first half-load goes out first so conv0's
            # first chunks start as early as possible; the (small) consts
            # follow on the same queue, then everything else
            x1_first = tiles[("x1", 0)]
            nc.gpsimd.dma_start(out=x1_first[0:P32, 0:64, :],
                                in_=x_d[0, :, 0:64, :])
            # consts go out on the idle Sync queue so they don't serialize
            # behind the x half-loads on gpsimd (and vice versa)
            nc.sync.dma_start(out=cma32[:, :], in_=cm32_d[:, :])
            nc.sync.dma_start(out=cma16[:, :], in_=cm16_d[:, :])
            nc.sync.dma_start(out=cma8[:, :], in_=cm8_d[:, :])
            nc.sync.dma_start(out=biasa[:, :], in_=bias_d[:, :])

            def conv32_block(blk, src, dst, post=None):
                """dst[0:P32,:,1:33] = relu(conv(src)+bias).  Generator:
                yields after each PSUM chunk so two sub-tiles can be emitted
                interleaved (fills PE stalls of one with the other's
                matmuls)."""
                for j in range(8):
                    b0, b1 = j * 16, (j + 1) * 16
                    pt = pspool.tile([P32, 512], f32, tag="pt0")
                    for kx in range(3):
                        nc.tensor.matmul(
                            pt[:, :], cm32_t[(blk, kx)],
                            src[0:P32, b0:b1, kx:kx + 32],
                            start=(kx == 0), stop=(kx == 2))
                    nc.scalar.activation(
                        out=dst[0:P32, b0:b1, 1:33],
                        in_=pt[:, :].rearrange("p (b w) -> p b w", w=32),
                        func=AFT.Relu, bias=bias_t[blk], scale=1.0)
                    if post is not None:
                        post(b0, b1)
                    yield

            def conv16_block(blk, src, dst, post=None):
                """16-stage conv: kx=0,1 fused in one K=112 matmul against
                src's shifted replica (partitions 56:112), kx=2 single."""
                for j in range(4):
                    b0, b1 = j * 32, (j + 1) * 32
                    pt = pspool.tile([P16, 512], f32, tag="pt16")
                    nc.tensor.matmul(
                        pt[:, :], cm16f_t[blk - 2],
                        src[0:2 * P16, b0:b1, 0:16], start=True, stop=False)
                    nc.tensor.matmul(
                        pt[:, :], cm16s_t[blk - 2],
                        src[0:P16, b0:b1, 2:18], start=False, stop=True)
                    nc.scalar.activation(
                        out=dst[0:P16, b0:b1, 1:17],
                        in_=pt[:, :].rearrange("p (b w) -> p b w", w=16),
                        func=AFT.Relu, bias=bias_t[blk], scale=1.0)
                    if post is not None:
                        post(b0, b1)
                    yield

            def conv8_block(blk, src, dst, post=None):
                """8-stage conv: all three kx fused in one K=72 matmul
                against src's two shifted replicas (partitions 24:72)."""
                for j in range(2):
                    b0, b1 = j * 64, (j + 1) * 64
                    pt = pspool.tile([P8, 512], f32, tag="pt8")
                    nc.tensor.matmul(
                        pt[:, :], cm8f_t[blk - 5],
                        src[0:3 * P8, b0:b1, 0:8], start=True, stop=True)
                    nc.scalar.activation(
                        out=dst[0:P8, b0:b1, 1:9],
                        in_=pt[:, :].rearrange("p (b w) -> p b w", w=8),
                        func=AFT.Relu, bias=bias_t[blk], scale=1.0)
                    if post is not None:
                        post(b0, b1)
                    yield

            def rep16(t, b0=0, b1=NB):
                # shifted replica for the fused kx=0,1 matmul: partitions
                # 56:112 hold src shifted one element left.  The copy is a
                # FLAT free-range shift (one contiguous descriptor per
                # partition, not a per-(b,w) 32B scatter); the b-boundary
                # wrap lands in column 17, which the fused matmul never
                # reads (cols 0:16).
                f0, f1 = b0 * 18, b1 * 18
                src = t[0:P16].rearrange("p b c -> p (b c)")
                dst = t[P16:2 * P16].rearrange("p b c -> p (b c)")
                nc.gpsimd.dma_start(out=dst[:, f0:f1 - 1], in_=src[:, f0 + 1:f1])

            def rep8(t, b0=0, b1=NB):
                # two shifted replicas for the fused kx=0,1,2 matmul (same
                # flat-shift trick; wrap columns 8/9 are never read)
                f0, f1 = b0 * 10, b1 * 10
                src = t[0:P8].rearrange("p b c -> p (b c)")
                d1 = t[P8:2 * P8].rearrange("p b c -> p (b c)")
                d2 = t[2 * P8:3 * P8].rearrange("p b c -> p (b c)")
                nc.gpsimd.dma_start(out=d1[:, f0:f1 - 1], in_=src[:, f0 + 1:f1])
                nc.gpsimd.dma_start(out=d2[:, f0:f1 - 2], in_=src[:, f0 + 2:f1])

            def subtile_stages(t_i):
                s = t_i % 2
                T = lambda tag: tiles[(tag, s)]
                # ---- load x sub-tile (host pre-permuted to the exact
                # SBUF layout, holes and pad columns pre-zeroed): one
                # contiguous dependency-free DMA ----
                x1 = T("x1")
                if t_i == 0:
                    # half 0 was issued in the preamble, ahead of the consts
                    nc.gpsimd.dma_start(out=x1[0:P32, 64:NB, :],
                                        in_=x_d[t_i, :, 64:NB, :])
                else:
                    nc.gpsimd.dma_start(out=x1[0:P32, :, :], in_=x_d[t_i, :, :, :])
                yield

                # ---- 32x32 stage ----
                x2 = T("a2")
                s12 = T("a12")
                yield from conv32_block(
                    0, x1, x2,
                    post=lambda b0, b1: nc.vector.tensor_add(
                        s12[0:P32, b0:b1], x1[0:P32, b0:b1], x2[0:P32, b0:b1]))
                yield
                x3 = T("a3")
                s123 = T("a2")
                yield from conv32_block(
                    1, s12, x3,
                    post=lambda b0, b1: nc.vector.tensor_add(
                        s123[0:P32, b0:b1], s12[0:P32, b0:b1], x3[0:P32, b0:b1]))
                yield
                # maxpool 32->16: w-pairs on DVE, h-half realign via DMA,
                # h-pairs max on DVE.  The chain runs in two 64-image
                # halves so its steps pipeline (DVE on half 1 while DMA
                # moves half 0) and conv2's first chunks start after the
                # half-chain instead of the whole ~10us chain.
                wp = T("wp")
                wph = T("wph")
                x4 = T("b4")
                s123v = s123[0:P32, :, 1:33].rearrange("p b (x two) -> p b x two", two=2)
                for hb in range(2):
                    b0, b1 = hb * 64, (hb + 1) * 64
                    nc.vector.tensor_max(wp[0:P32, b0:b1, :],
                                         s123v[:, b0:b1, :, 0],
                                         s123v[:, b0:b1, :, 1])
                    nc.gpsimd.dma_start(out=wph[0:P16, b0:b1, :],
                                        in_=wp[64:120, b0:b1, :])
                    nc.vector.tensor_max(x4[0:P16, b0:b1, 1:17],
                                         wp[0:P16, b0:b1, :],
                                         wph[0:P16, b0:b1, :])
                    rep16(x4, b0, b1)
                    yield

                # ---- 16x16 stage ----
                x5 = T("b5")
                s45 = T("b45")
                yield from conv16_block(
                    2, x4, x5,
                    post=lambda b0, b1: nc.vector.tensor_add(
                        s45[0:P16, b0:b1], x4[0:P16, b0:b1], x5[0:P16, b0:b1]))
                rep16(s45)
                yield
                x6 = T("b6")
                t56 = T("b56")
                s456 = T("b4")
                def post3(b0, b1):
                    nc.vector.tensor_add(
                        t56[0:P16, b0:b1], x5[0:P16, b0:b1], x6[0:P16, b0:b1])
                    nc.vector.tensor_add(
                        s456[0:P16, b0:b1], s45[0:P16, b0:b1], x6[0:P16, b0:b1])
                yield from conv16_block(3, s45, x6, post=post3)
                rep16(s456)
                yield
                x7 = T("b7")
                s567 = T("b45")
                yield from conv16_block(
                    4, s456, x7,
                    post=lambda b0, b1: nc.vector.tensor_add(
                        s567[0:P16, b0:b1], t56[0:P16, b0:b1], x7[0:P16, b0:b1]))
                yield
                wp2 = T("wp2")
                wph2 = T("wph2")
                x8 = T("c8")
                s567v = s567[0:P16, :, 1:17].rearrange("p b (x two) -> p b x two", two=2)
                for hb in range(2):
                    b0, b1 = hb * 64, (hb + 1) * 64
                    nc.vector.tensor_max(wp2[0:P16, b0:b1, :],
                                         s567v[:, b0:b1, :, 0],
                                         s567v[:, b0:b1, :, 1])
                    nc.gpsimd.dma_start(out=wph2[0:P8, b0:b1, :],
                                        in_=wp2[32:56, b0:b1, :])
                    nc.vector.tensor_max(x8[0:P8, b0:b1, 1:9],
                                         wp2[0:P8, b0:b1, :],
                                         wph2[0:P8, b0:b1, :])
                    rep8(x8, b0, b1)
                    yield

                # ---- 8x8 stage ----
                x9 = T("c9")
                s89 = T("c89")
                yield from conv8_block(
                    5, x8, x9,
                    post=lambda b0, b1: nc.vector.tensor_add(
                        s89[0:P8, b0:b1], x8[0:P8, b0:b1], x9[0:P8, b0:b1]))
                rep8(s89)
                yield
                x10 = T("c10")
                s8910 = T("c9")
                yield from conv8_block(
                    6, s89, x10,
                    post=lambda b0, b1: nc.vector.tensor_add(
                        s8910[0:P8, b0:b1], s89[0:P8, b0:b1], x10[0:P8, b0:b1]))
                rep8(s8910)
                yield
                x11 = T("c10")
                # ---- GAP folded into blk7's chunk loop ----
                gsum = spool.tile([P8, NB], f32, tag="g")
                yield from conv8_block(
                    7, s8910, x11,
                    post=lambda b0, b1: nc.vector.reduce_sum(
                        out=gsum[:, b0:b1], in_=x11[0:P8, b0:b1, 1:9], axis=AX.X))
                yield
                ph = phpool.tile([128, 10], f32, tag="ph")
                nc.tensor.matmul(ph[:, :], gsum[:, :], gh_t, start=True, stop=True)
                mx = spool.tile([128, 1], f32, tag="m")
                nc.vector.reduce_max(out=mx[:, :], in_=ph[:, :], axis=AX.X)
                negm = spool.tile([128, 1], f32, tag="negm")
                nc.vector.tensor_scalar_mul(negm[:, :], mx[:, :], -1.0)
                yield
                ex = spool.tile([128, 10], f32, tag="e")
                ssum = spool.tile([128, 1], f32, tag="ssum")
                nc.scalar.activation(
                    out=ex[:, :], in_=ph[:, :], func=AFT.Exp,
                    bias=negm[:, :], scale=1.0, accum_out=ssum[:, :])
                ls = spool.tile([128, 1], f32, tag="ls")
                nc.scalar.activation(out=ls[:, :], in_=ssum[:, :], func=AFT.Ln)
                nc.vector.tensor_scalar(
                    out=res_all[:, t_i, :], in0=ph[:, :], scalar1=negm[:, :],
                    scalar2=ls[:, :], op0=ALU.add, op1=ALU.subtract)
                yield

            # Sub-tile pairs run sequentially (pair B after pair A), but
            # WITHIN a pair the twin starts 5 rounds late.  Engine queues
            # are in-order, so a PE gap can only be filled by work emitted
            # at that queue position: in lockstep the twins hit their pool
            # chains (w-max -> realign DMA -> h-max -> rep DMA) at the
            # same time and stall together; offset, the twin's still-ready
            # conv chunks sit at exactly the stalled slots.  Twins share
            # no buffers, so any offset is correctness-free; pair B must
            # still emit strictly after pair A (shared buffer sets, and
            # emission order is program order for the dep tracker).
            O = 5
            start_round = {0: 0, 1: O, 2: 10 ** 6, 3: 10 ** 6 + O}
            gens = {k: subtile_stages(k) for k in range(N_SUB)}
            active = []
            round_i = 0
            while gens or active:
                for k in sorted(list(gens)):
                    if round_i >= start_round[k]:
                        active.append(gens.pop(k))
                if not active and gens:
                    # pair A fully emitted: let pair B in
                    round_i = 10 ** 6
                    continue
                active = [g for g in active if next(g, 1) is None]
                round_i += 1

            # single output DMA
            dst = bass.AP(tensor=out_d, offset=0,
                          ap=[[10, 128], [NB * 10, N_SUB], [1, 10]])
            nc.sync.dma_start(out=dst, in_=res_all[:, :, :])

    return nc


def _prep_x(shard):
    """[B_CORE,3,32,32] -> [N_SUB,128,NB,34] bf16 in the kernel's SBUF
    layout (h-permuted partitions, zero pool-hole rows, zero w-pad cols)."""
    import ml_dtypes
    xs = shard.reshape(N_SUB, NB, 3, 32, 32)
    xp = np.zeros((N_SUB, P32, NB, 34), ml_dtypes.bfloat16)
    for c in range(3):
        for h in range(32):
            xp[:, _rmap32(c, h), :, 1:33] = xs[:, :, c, h, :].astype(
                ml_dtypes.bfloat16)
    return np.ascontiguousarray(xp)


def _make_in_maps(x, consts):
    x = np.ascontiguousarray(np.asarray(x, np.float32))
    in_maps = []
    for i in range(N_CORES):
        shard = x[i * B_CORE:(i + 1) * B_CORE]
        m = {"x": _prep_x(shard)}
        m.update(consts)
        in_maps.append(m)
    return in_maps


_PATCHED = False


def _split_multiwait(bir_json):
    """Walrus in this container accepts at most ONE sem-wait per
    instruction (setupSyncWait: 'Too many sync wait commands').  Tile's
    scheduler freely emits several.  Split the extras into single-wait
    EventSemaphore instructions on the same engine, immediately before the
    original instruction — same queue, so the sequencer performs the waits
    in order before issuing it."""
    import json
    d = json.loads(bir_json)
    cnt = 0
    for fn in d.get("functions", []):
        bkey = "basic_blocks" if "basic_blocks" in fn else "blocks"
        for blk in fn.get(bkey, []):
            out = []
            for inst in blk["instructions"]:
                si = inst.get("sync_info")
                ws = (si or {}).get("on_wait") or []
                if len(ws) > 1:
                    for w in ws[:-1]:
                        cnt += 1
                        out.append({
                            "debug": inst.get("debug", 0),
                            "engine": inst["engine"],
                            "ins": [], "outs": [],
                            "name": f"swsplit_{cnt}",
                            "opcode": "EventSemaphore",
                            "sync_info": {"on_wait": [w], "on_update": []},
                        })
                    si["on_wait"] = [ws[-1]]
                out.append(inst)
            blk["instructions"] = out
    return json.dumps(d).encode()


def _install_compile_patch():
    global _PATCHED
    if _PATCHED:
        return
    import concourse.bass_utils as _bu
    import concourse.bass2jax as _b2j

    orig = _bu.compile_bir_kernel

    def patched(bir_json, tmpdir, neff_name="file.neff"):
        return orig(_split_multiwait(bir_json), tmpdir, neff_name)

    _bu.compile_bir_kernel = patched
    _b2j.compile_bir_kernel = patched
    _PATCHED = True


def run(x, consts, trace=False):
    from concourse.bass_utils import run_bass_kernel_spmd

    _install_compile_patch()
    nc = build_program()
    res = run_bass_kernel_spmd(
        nc, _make_in_maps(x, consts), list(range(N_CORES)), trace=trace)
    out = np.concatenate([res.results[i]["out"] for i in range(N_CORES)], axis=0)
    return out, res


def time_warm(x, consts, iters=10):
    """Time warm executions of the compiled NEFF across all 8 cores.

    Rebuilds the pjrt callable (NEFF comes from the compile cache), keeps
    inputs resident on device, and times repeated dispatches."""
    import time
    import jax
    from jax.sharding import Mesh, PartitionSpec, NamedSharding
    from jax.experimental.shard_map import shard_map
    from concourse import bass2jax, mybir

    _install_compile_patch()
    nc = build_program()
    bass2jax.install_neuronx_cc_hook()
    in_maps = _make_in_maps(x, consts)

    partition_name = (nc.partition_id_tensor.name
                      if nc.partition_id_tensor else None)
    in_names, out_names, out_avals, zero_outs = [], [], [], []
    for alloc in nc.m.functions[0].allocations:
        if not isinstance(alloc, mybir.MemoryLocationSet):
            continue
        name = alloc.memorylocations[0].name
        if alloc.kind == "ExternalInput":
            if name != partition_name:
                in_names.append(name)
        elif alloc.kind == "ExternalOutput":
            shape = tuple(alloc.tensor_shape)
            dtype = mybir.dt.np(alloc.dtype)
            out_names.append(name)
            out_avals.append(jax.core.ShapedArray(shape, dtype))
            zero_outs.append(np.zeros(shape, dtype))
    n_params = len(in_names)
    n_outs = len(out_names)
    all_names = in_names + out_names
    if partition_name is not None:
        all_names = all_names + [partition_name]
    donate = tuple(range(n_params, n_params + n_outs))

    def _body(*args):
        operands = list(args)
        if partition_name is not None:
            operands.append(bass2jax.partition_id_tensor())
        outs = bass2jax._bass_exec_p.bind(
            *operands,
            out_avals=tuple(out_avals),
            in_names=tuple(all_names),
            out_names=tuple(out_names),
            lowering_input_output_aliases=(),
            sim_require_finite=True,
            sim_require_nnan=True,
            nc=nc,
        )
        return tuple(outs)

    devices = jax.devices()[:N_CORES]
    mesh = Mesh(np.asarray(devices), ("core",))
    in_specs = (PartitionSpec("core"),) * (n_params + n_outs)
    out_specs = (PartitionSpec("core"),) * n_outs
    sharded = jax.jit(
        shard_map(_body, mesh=mesh, in_specs=in_specs, out_specs=out_specs,
                  check_rep=False),
        donate_argnums=donate, keep_unused=True)

    sh = NamedSharding(mesh, PartitionSpec("core"))
    concat_in = [
        jax.device_put(
            np.concatenate([np.asarray(in_maps[c][name]) for c in range(N_CORES)],
                           axis=0), sh)
        for name in in_names
    ]
    for a in concat_in:
        a.block_until_ready()

    def zeros():
        return [np.zeros((N_CORES * z.shape[0], *z.shape[1:]), z.dtype)
                for z in zero_outs]

    r = sharded(*concat_in, *zeros())  # warmup (compile-cache hit)
    jax.block_until_ready(r)
    # serial (includes full dispatch round-trip each call)
    best = float("inf")
    for _ in range(iters):
        zs = zeros()
        t0 = time.perf_counter()
        r = sharded(*concat_in, *zs)
        jax.block_until_ready(r)
        best = min(best, time.perf_counter() - t0)
    # pipelined back-to-back dispatches amortize the RPC round-trip
    n_pipe = 20
    zss = [zeros() for _ in range(n_pipe)]
    t0 = time.perf_counter()
    rs = [sharded(*concat_in, *z) for z in zss]
    jax.block_until_ready(rs)
    pipe = (time.perf_counter() - t0) / n_pipe
    return min(best, pipe) * 1e9


def kernel(x, ws, w9, gammas, betas, means, variances):
    consts = _build_consts(ws, w9, gammas, betas, means, variances)
    out, _ = run(x, consts, trace=False)
    return np.asarray(out, np.float32)

